# revision 14
# baseline (speedup 1.0000x reference)
"""Trainium2 Bass kernel for MultiHeadSyntonicAttention.

Problem: B=2, S=2048, D=1024, H=16 heads, DH=64.
  q/k/v = Linear(query/key/value); per-head gnosis gate
  gn = sigmoid(k . wg + bg); scores = (q k^T / sqrt(dh)) * (1+gn);
  out = softmax(scores) v;  out = ((out Wo+bo) Wd+bd) Wh+bh.

Sharding (8 cores): core c -> batch b=c//4, head-group g=c%4 (4 heads).
Each core computes its heads' attention and a row-slice partial of the
fused output projection Wf = Wo@Wd@Wh; host sums 4 partials per batch.

Device layout (everything "transposed", tokens on the free axis):
  host ships qT/kT/vT = x[b].T  [D=1024, S=2048] bf16
  QT[c,s] (c = head-local dim, 256 rows) = Wq_s^T qT     (lhsT=Wq_s nat.)
  K gating folded into K: K' = KT * (1+gn[head, s])
  ST[k,q] = K'T^T Q                      (contract dh=64)
  PT = exp(ST/8)  bf16                   (max-sub skipped; scores bounded)
  OT[0:64,q] accum over k-blocks: lhsT=[V_head | ones] -> row 64 = denom
  ctxT = OT[0:64]/OT[64]  -> partial = ctxT^T Wf_s  [2048, 1024] f32

PSUM is 8 banks x 512 f32; attention runs in q-halves of 1024 so score
tiles (2 banks) and PV accumulators (2 banks) double-buffer within 8.
"""

import sys

sys.path.insert(0, "/opt/trn_rl_repo")

import numpy as np
import ml_dtypes

BF16 = ml_dtypes.bfloat16

B, S, D, H = 2, 2048, 1024, 16
DH = D // H          # 64
HPC = 4              # heads per core
C = HPC * DH         # 256 head-local columns per core
NCORES = 8
ND = D // 128        # 8 d-chunks
NSB = S // 128       # 16 s-blocks
QW = 1024            # attention q-tile width
NQH = S // QW        # 2 q-halves

_nc_cache = {}


def build_bass():
    import concourse.bass as bass
    import concourse.mybir as mybir
    import concourse.tile as tile
    from concourse import bacc

    f32 = mybir.dt.float32
    bf16 = mybir.dt.bfloat16
    Alu = mybir.AluOpType
    Act = mybir.ActivationFunctionType

    nc = bacc.Bacc(None, target_bir_lowering=False, name="syntonic_attn")

    qT_d = nc.dram_tensor("qT", [D, S], bf16, kind="ExternalInput")
    kT_d = nc.dram_tensor("kT", [D, S], bf16, kind="ExternalInput")
    vT_d = nc.dram_tensor("vT", [D, S], bf16, kind="ExternalInput")
    wq_d = nc.dram_tensor("wq", [D, C], bf16, kind="ExternalInput")
    wk_d = nc.dram_tensor("wk", [D, C], bf16, kind="ExternalInput")
    wv_d = nc.dram_tensor("wv", [D, C], bf16, kind="ExternalInput")
    wf_d = nc.dram_tensor("wf", [C, D], bf16, kind="ExternalInput")
    wg4_d = nc.dram_tensor("wg4", [C, HPC], bf16, kind="ExternalInput")
    bq_d = nc.dram_tensor("bq", [1, C], bf16, kind="ExternalInput")
    bk_d = nc.dram_tensor("bk", [1, C], bf16, kind="ExternalInput")
    bv_d = nc.dram_tensor("bv", [1, C], bf16, kind="ExternalInput")
    bg_d = nc.dram_tensor("bg128", [128, 1], f32, kind="ExternalInput")
    out_d = nc.dram_tensor("out", [S, D], f32, kind="ExternalOutput")

    with tile.TileContext(nc) as tc:
        with (
            tc.tile_pool(name="res", bufs=1) as res,       # resident inputs
            tc.tile_pool(name="acts", bufs=1) as acts,     # projected acts
            tc.tile_pool(name="work", bufs=3) as work,     # PT tiles
            tc.tile_pool(name="outp", bufs=3) as outp,     # out staging
            tc.tile_pool(name="psum", bufs=1, space="PSUM") as psum,
        ):
            # ---------------- resident input loads ----------------
            qT = [res.tile([128, S], bf16, tag=f"qT{i}", name=f"qT{i}") for i in range(ND)]
            kT = [res.tile([128, S], bf16, tag=f"kT{i}", name=f"kT{i}") for i in range(ND)]
            vT = [res.tile([128, S], bf16, tag=f"vT{i}", name=f"vT{i}") for i in range(ND)]
            wq = [res.tile([128, C], bf16, tag=f"wq{i}", name=f"wq{i}") for i in range(ND)]
            wk = [res.tile([128, C], bf16, tag=f"wk{i}", name=f"wk{i}") for i in range(ND)]
            wv = [res.tile([128, C], bf16, tag=f"wv{i}", name=f"wv{i}") for i in range(ND)]
            wf = [res.tile([128, D], bf16, tag=f"wf{i}", name=f"wf{i}") for i in range(2)]
            wg4 = [res.tile([128, HPC], bf16, tag=f"wg4{i}", name=f"wg4{i}") for i in range(2)]
            bq = res.tile([1, C], bf16, tag="bq")
            bk = res.tile([1, C], bf16, tag="bk")
            bv = res.tile([1, C], bf16, tag="bv")
            bg128 = res.tile([128, 1], f32, tag="bg128")
            ones = res.tile([1, 512], bf16, tag="ones")
            c8 = res.tile([128, 1], f32, tag="c8")

            for i in range(ND):
                nc.sync.dma_start(wq[i][:], wq_d[i * 128:(i + 1) * 128, :])
                nc.sync.dma_start(qT[i][:], qT_d[i * 128:(i + 1) * 128, :])
            for i in range(ND):
                nc.sync.dma_start(wk[i][:], wk_d[i * 128:(i + 1) * 128, :])
                nc.sync.dma_start(kT[i][:], kT_d[i * 128:(i + 1) * 128, :])
            for i in range(ND):
                nc.sync.dma_start(wv[i][:], wv_d[i * 128:(i + 1) * 128, :])
                nc.sync.dma_start(vT[i][:], vT_d[i * 128:(i + 1) * 128, :])
            wg4c = [res.tile([128, HPC], bf16, tag=f"wg4c{i}", name=f"wg4c{i}")
                    for i in range(2)]
            for i in range(2):
                nc.sync.dma_start(wf[i][:], wf_d[i * 128:(i + 1) * 128, :])
                nc.sync.dma_start(wg4[i][:], wg4_d[i * 128:(i + 1) * 128, :])
                nc.vector.tensor_copy(wg4c[i][:], wg4[i][:])
            nc.sync.dma_start(bq[:], bq_d[:])
            nc.sync.dma_start(bk[:], bk_d[:])
            nc.sync.dma_start(bv[:], bv_d[:])
            nc.sync.dma_start(bg128[:], bg_d[:])
            nc.vector.memset(ones[:], 1.0)
            nc.vector.memset(c8[:], 0.125)

            # ---------------- Q/K projections (transposed out) ----------------
            # XT[c, s] = sum_d W[d, c] * xT[d, s]  (+ bias[c] via rank-1)
            QT = [acts.tile([128, S], bf16, tag=f"QT{i}", name=f"QT{i}") for i in range(2)]
            KT = [acts.tile([128, S], bf16, tag=f"KT{i}", name=f"KT{i}") for i in range(2)]
            ctxT = [acts.tile([128, S], bf16, tag=f"ctxT{i}", name=f"ctxT{i}") for i in range(2)]

            def project_T(w_tiles, x_tiles, bias, dest, cb, qh, pname):
                q0 = qh * QW
                ps = psum.tile([128, QW], f32, tag=f"sth{cb}", name=pname)
                for dc in range(ND):
                    lhsT = w_tiles[dc][:, cb * 128:(cb + 1) * 128]
                    for qc in range(QW // 512):
                        nc.tensor.matmul(
                            ps[:, qc * 512:(qc + 1) * 512],
                            lhsT,
                            x_tiles[dc][:, q0 + qc * 512:q0 + (qc + 1) * 512],
                            start=(dc == 0),
                            stop=False,
                        )
                for qc in range(QW // 512):
                    nc.tensor.matmul(
                        ps[:, qc * 512:(qc + 1) * 512],
                        bias[0:1, cb * 128:(cb + 1) * 128],
                        ones[0:1, :],
                        start=False,
                        stop=True,
                    )
                nc.vector.tensor_copy(dest[:, q0:q0 + QW], ps[:])

            for cb in range(2):
                for qh in range(NQH):
                    project_T(wq, qT, bq, QT[cb], cb, qh, f"psq{cb}{qh}")
            for cb in range(2):
                for qh in range(NQH):
                    project_T(wk, kT, bk, KT[cb], cb, qh, f"psk{cb}{qh}")

            # ---------------- gnosis gates (transposed) ----------------
            # glinT[s, h] = sum_c KT[c, s] * wg4[c, h];
            # gscT = (1 + sigmoid(glinT + bg)) / 8  -> per-partition exp scale
            gscT = [acts.tile([128, HPC], f32, tag=f"gsc{i}", name=f"gsc{i}")
                    for i in range(NSB)]
            for sb in range(NSB):
                gps = psum.tile([128, HPC], f32, tag="oth1", name=f"gps{sb}")
                for cc in range(2):
                    nc.tensor.matmul(
                        gps[:],
                        KT[cc][:, sb * 128:(sb + 1) * 128],
                        wg4c[cc][:],
                        start=(cc == 0),
                        stop=(cc == 1),
                    )
                gn = work.tile([128, HPC], f32, tag="gn", name=f"gn{sb}", bufs=2)
                nc.scalar.activation(gn[:], gps[:], Act.Sigmoid, bias=bg128[:], scale=1.0)
                # (1+gn)/8 on ACT too: keeps the exp's scale dep same-engine
                nc.scalar.activation(
                    gscT[sb][:], gn[:], Act.Identity, bias=c8[:], scale=c8[:]
                )

            # ---------------- V projection (natural layout + ones block) ----------------
            # V[s, c] tiles [128, 4*128]: head h at cols h*128..+63 = V, cols
            # h*128+64..+127 = 1.0 -> PV matmul rows 64:128 replicate the
            # softmax denominator across 64 partitions (free normalization).
            V = [acts.tile([128, HPC * 2 * DH], bf16, tag=f"V{i}", name=f"V{i}")
                 for i in range(NSB)]
            for sb in range(NSB):
                nc.vector.memset(V[sb][:], 1.0)
                ps = psum.tile([128, C], f32, tag="oth0", name=f"psv{sb}")
                for dc in range(ND):
                    nc.tensor.matmul(
                        ps[:],
                        vT[dc][:, sb * 128:(sb + 1) * 128],
                        wv[dc][:],
                        start=(dc == 0),
                        stop=False,
                    )
                nc.tensor.matmul(
                    ps[:], ones[0:1, 0:128], bv[:], start=False, stop=True
                )
                nc.vector.tensor_copy(
                    V[sb][:].rearrange("p (h x) -> p h x", h=HPC)[:, :, 0:DH],
                    ps[:].rearrange("p (h x) -> p h x", h=HPC),
                )

            # ---------------- attention: head pairs interleaved ----------------
            # Pair (h0, h1) = (2cb, 2cb+1) live on partitions 0:64 / 64:128 of
            # QT/KT tile cb -> their QK matmuls target different PE row groups
            # and run concurrently (LDWEIGHTS hidden by the reorder window).
            for cb in range(2):
                h0, h1 = 2 * cb, 2 * cb + 1
                for qh in range(NQH):
                    q0 = qh * QW
                    ota = psum.tile([128, QW], f32, tag="oth0", name=f"ota{cb}{qh}")
                    otb = psum.tile([128, QW], f32, tag="oth1", name=f"otb{cb}{qh}")
                    for kb in range(NSB):
                        sta = psum.tile([128, QW], f32, tag="sth0", name=f"sta{cb}{qh}{kb}")
                        stb = psum.tile([128, QW], f32, tag="sth1", name=f"stb{cb}{qh}{kb}")
                        kslc = slice(kb * 128, (kb + 1) * 128)
                        for qc in range(QW // 512):
                            cs = slice(qc * 512, (qc + 1) * 512)
                            qs = slice(q0 + qc * 512, q0 + (qc + 1) * 512)
                            nc.tensor.matmul(sta[:, cs], KT[cb][0:64, kslc],
                                             QT[cb][0:64, qs], start=True, stop=True)
                            nc.tensor.matmul(stb[:, cs], KT[cb][64:128, kslc],
                                             QT[cb][64:128, qs], start=True, stop=True)
                        pta = work.tile([128, QW], bf16, tag="pta", name=f"pta{cb}{qh}{kb}", bufs=2)
                        ptb = work.tile([128, QW], bf16, tag="ptb", name=f"ptb{cb}{qh}{kb}", bufs=2)
                        nc.scalar.activation(pta[:], sta[:], Act.Exp,
                                             scale=gscT[kb][:, h0:h0 + 1])
                        nc.scalar.activation(ptb[:], stb[:], Act.Exp,
                                             scale=gscT[kb][:, h1:h1 + 1])
                        vh0 = V[kb][:, h0 * 2 * DH:(h0 + 1) * 2 * DH]
                        vh1 = V[kb][:, h1 * 2 * DH:(h1 + 1) * 2 * DH]
                        for qc in range(QW // 512):
                            cs = slice(qc * 512, (qc + 1) * 512)
                            nc.tensor.matmul(ota[:, cs], vh0, pta[:, cs],
                                             start=(kb == 0), stop=(kb == NSB - 1))
                        for qc in range(QW // 512):
                            cs = slice(qc * 512, (qc + 1) * 512)
                            nc.tensor.matmul(otb[:, cs], vh1, ptb[:, cs],
                                             start=(kb == 0), stop=(kb == NSB - 1))
                    # evacuate PSUM promptly (releases slots for next iter),
                    # then normalize on SBUF off the PE critical path
                    for po, ot in ((0, ota), (64, otb)):
                        otc = work.tile([128, QW], f32, tag=f"otc{po}", name=f"otc{cb}{qh}{po}", bufs=1)
                        nc.vector.tensor_copy(otc[:], ot[:])
                        rec = work.tile([DH, QW], f32, tag=f"rec{po}", name=f"rec{cb}{qh}{po}", bufs=1)
                        nc.vector.reciprocal(rec[:], otc[DH:2 * DH, :])
                        nc.vector.tensor_tensor(
                            ctxT[cb][po:po + 64, q0:q0 + QW],
                            otc[0:DH, :],
                            rec[:],
                            Alu.mult,
                        )

            # ---------------- fused output projection ----------------
            # partial[s, o] = sum_c ctxT[c, s] * Wf[c, o]
            for qb in range(NSB):
                ps0 = psum.tile([128, 512], f32, tag="oth0", name=f"pso{qb}a")
                ps1 = psum.tile([128, 512], f32, tag="oth1", name=f"pso{qb}b")
                for cc in range(2):
                    lhsT = ctxT[cc][:, qb * 128:(qb + 1) * 128]
                    for oc, ps in enumerate((ps0, ps1)):
                        nc.tensor.matmul(
                            ps[:],
                            lhsT,
                            wf[cc][:, oc * 512:(oc + 1) * 512],
                            start=(cc == 0),
                            stop=(cc == 1),
                        )
                ob = outp.tile([128, D], f32, tag="ob", name=f"ob{qb}")
                nc.vector.tensor_copy(ob[:, 0:512], ps0[:])
                nc.vector.tensor_copy(ob[:, 512:1024], ps1[:])
                nc.sync.dma_start(out_d[qb * 128:(qb + 1) * 128, :], ob[:])

    nc.finalize()
    return nc


def get_nc():
    if "nc" not in _nc_cache:
        _nc_cache["nc"] = build_bass()
    return _nc_cache["nc"]


def make_in_maps(query, key_, value, Wq, bq, Wk, bk, Wv, bv, wg, bg, Wo, bo, Wd, bd, Wh, bh):
    """Host-side sharding: returns (in_maps for 8 cores, fused bias)."""
    f = np.asarray
    Wf = f(Wo, np.float64) @ f(Wd, np.float64) @ f(Wh, np.float64)
    bf = (f(bo, np.float64) @ f(Wd, np.float64) @ f(Wh, np.float64)
          + f(bd, np.float64) @ f(Wh, np.float64) + f(bh, np.float64))

    wg4 = np.zeros((C, HPC), np.float32)
    for h in range(HPC):
        wg4[h * DH:(h + 1) * DH, h] = np.asarray(wg, np.float32)
    wg4 = wg4.astype(BF16)
    bg128 = np.full((128, 1), np.float32(bg), np.float32)

    xT = []
    for b in range(B):
        xT.append(tuple(
            np.ascontiguousarray(np.asarray(x[b], np.float32).T).astype(BF16)
            for x in (query, key_, value)
        ))

    in_maps = []
    for c in range(NCORES):
        b, g = divmod(c, HPC)
        cols = slice(g * C, (g + 1) * C)
        qTb, kTb, vTb = xT[b]
        in_maps.append({
            "qT": qTb, "kT": kTb, "vT": vTb,
            "wq": np.ascontiguousarray(np.asarray(Wq, np.float32)[:, cols]).astype(BF16),
            "wk": np.ascontiguousarray(np.asarray(Wk, np.float32)[:, cols]).astype(BF16),
            "wv": np.ascontiguousarray(np.asarray(Wv, np.float32)[:, cols]).astype(BF16),
            "wf": np.ascontiguousarray(Wf[cols, :]).astype(BF16),
            "wg4": wg4, "bg128": bg128,
            "bq": np.asarray(bq, np.float32)[None, cols].astype(BF16),
            "bk": np.asarray(bk, np.float32)[None, cols].astype(BF16),
            "bv": np.asarray(bv, np.float32)[None, cols].astype(BF16),
        })
    return in_maps, bf.astype(np.float32)


def gather(results, bf):
    out = np.zeros((B, S, D), np.float32)
    for c in range(NCORES):
        b = c // HPC
        out[b] += results[c]["out"]
    out += bf[None, None, :]
    return out


def kernel(**inputs):
    from concourse.bass_utils import run_bass_kernel_spmd

    nc = get_nc()
    in_maps, bf = make_in_maps(**inputs)
    res = run_bass_kernel_spmd(nc, in_maps, core_ids=list(range(NCORES)))
    return gather(res.results, bf)


# revision 15
# speedup vs baseline: 1.3118x; 1.3118x over previous
"""Trainium2 Bass kernel for MultiHeadSyntonicAttention.

Problem: B=2, S=2048, D=1024, H=16 heads, DH=64.
  q/k/v = Linear(query/key/value); per-head gnosis gate
  gn = sigmoid(k . wg + bg); scores = (q k^T / sqrt(dh)) * (1+gn);
  out = softmax(scores) v;  out = ((out Wo+bo) Wd+bd) Wh+bh.

Sharding (8 cores): core c -> batch b=c//4, head-group g=c%4 (4 heads).
Each core computes its heads' attention and a row-slice partial of the
fused output projection Wf = Wo@Wd@Wh; host sums 4 partials per batch.

Device layout (everything "transposed", tokens on the free axis):
  host ships qT/kT/vT = x[b].T  [D=1024, S=2048] bf16
  QT[c,s] (c = head-local dim, 256 rows) = Wq_s^T qT     (lhsT=Wq_s nat.)
  K gating folded into K: K' = KT * (1+gn[head, s])
  ST[k,q] = K'T^T Q                      (contract dh=64)
  PT = exp(ST/8)  bf16                   (max-sub skipped; scores bounded)
  OT[0:64,q] accum over k-blocks: lhsT=[V_head | ones] -> row 64 = denom
  ctxT = OT[0:64]/OT[64]  -> partial = ctxT^T Wf_s  [2048, 1024] f32

PSUM is 8 banks x 512 f32; attention runs in q-halves of 1024 so score
tiles (2 banks) and PV accumulators (2 banks) double-buffer within 8.
"""

import sys

sys.path.insert(0, "/opt/trn_rl_repo")

import numpy as np
import ml_dtypes

BF16 = ml_dtypes.bfloat16

B, S, D, H = 2, 2048, 1024, 16
DH = D // H          # 64
HPC = 4              # heads per core
C = HPC * DH         # 256 head-local columns per core
NCORES = 8
ND = D // 128        # 8 d-chunks
NSB = S // 128       # 16 s-blocks
QW = 1024            # attention q-tile width
NQH = S // QW        # 2 q-halves

_nc_cache = {}


def build_bass():
    import concourse.bass as bass
    import concourse.mybir as mybir
    import concourse.tile as tile
    from concourse import bacc

    f32 = mybir.dt.float32
    bf16 = mybir.dt.bfloat16
    Alu = mybir.AluOpType
    Act = mybir.ActivationFunctionType

    nc = bacc.Bacc(None, target_bir_lowering=False, name="syntonic_attn")

    qT_d = nc.dram_tensor("qT", [D, S], bf16, kind="ExternalInput")
    kT_d = nc.dram_tensor("kT", [D, S], bf16, kind="ExternalInput")
    vT_d = nc.dram_tensor("vT", [D, S], bf16, kind="ExternalInput")
    wq_d = nc.dram_tensor("wq", [D, C], bf16, kind="ExternalInput")
    wk_d = nc.dram_tensor("wk", [D, C], bf16, kind="ExternalInput")
    wv_d = nc.dram_tensor("wv", [D, C], bf16, kind="ExternalInput")
    wf_d = nc.dram_tensor("wf", [C, D], bf16, kind="ExternalInput")
    wg4_d = nc.dram_tensor("wg4", [C, HPC], bf16, kind="ExternalInput")
    bq_d = nc.dram_tensor("bq", [1, C], bf16, kind="ExternalInput")
    bk_d = nc.dram_tensor("bk", [1, C], bf16, kind="ExternalInput")
    bv_d = nc.dram_tensor("bv", [1, C], bf16, kind="ExternalInput")
    bg_d = nc.dram_tensor("bg128", [128, 1], f32, kind="ExternalInput")
    out_d = nc.dram_tensor("out", [S, D], f32, kind="ExternalOutput")

    with tile.TileContext(nc) as tc:
        with (
            tc.tile_pool(name="res", bufs=1) as res,       # resident inputs
            tc.tile_pool(name="acts", bufs=1) as acts,     # projected acts
            tc.tile_pool(name="work", bufs=3) as work,     # PT tiles
            tc.tile_pool(name="outp", bufs=3) as outp,     # out staging
            tc.tile_pool(name="psum", bufs=1, space="PSUM") as psum,
        ):
            # ---------------- resident input loads ----------------
            qT = [res.tile([128, S], bf16, tag=f"qT{i}", name=f"qT{i}") for i in range(ND)]
            kT = [res.tile([128, S], bf16, tag=f"kT{i}", name=f"kT{i}") for i in range(ND)]
            vT = [res.tile([128, S], bf16, tag=f"vT{i}", name=f"vT{i}") for i in range(ND)]
            wq = [res.tile([128, C], bf16, tag=f"wq{i}", name=f"wq{i}") for i in range(ND)]
            wk = [res.tile([128, C], bf16, tag=f"wk{i}", name=f"wk{i}") for i in range(ND)]
            wv = [res.tile([128, C], bf16, tag=f"wv{i}", name=f"wv{i}") for i in range(ND)]
            wf = [res.tile([128, D], bf16, tag=f"wf{i}", name=f"wf{i}") for i in range(2)]
            wg4 = [res.tile([128, HPC], bf16, tag=f"wg4{i}", name=f"wg4{i}") for i in range(2)]
            bq = res.tile([1, C], bf16, tag="bq")
            bk = res.tile([1, C], bf16, tag="bk")
            bv = res.tile([1, C], bf16, tag="bv")
            bg128 = res.tile([128, 1], f32, tag="bg128")
            ones = res.tile([1, 512], bf16, tag="ones")
            c8 = res.tile([128, 1], f32, tag="c8")

            for i in range(ND):
                nc.sync.dma_start(wq[i][:], wq_d[i * 128:(i + 1) * 128, :])
                nc.sync.dma_start(qT[i][:], qT_d[i * 128:(i + 1) * 128, :])
            for i in range(ND):
                nc.sync.dma_start(wk[i][:], wk_d[i * 128:(i + 1) * 128, :])
                nc.sync.dma_start(kT[i][:], kT_d[i * 128:(i + 1) * 128, :])
            for i in range(ND):
                nc.sync.dma_start(wv[i][:], wv_d[i * 128:(i + 1) * 128, :])
                nc.sync.dma_start(vT[i][:], vT_d[i * 128:(i + 1) * 128, :])
            wg4c = [res.tile([128, HPC], bf16, tag=f"wg4c{i}", name=f"wg4c{i}")
                    for i in range(2)]
            for i in range(2):
                nc.sync.dma_start(wf[i][:], wf_d[i * 128:(i + 1) * 128, :])
                nc.sync.dma_start(wg4[i][:], wg4_d[i * 128:(i + 1) * 128, :])
                nc.vector.tensor_copy(wg4c[i][:], wg4[i][:])
            nc.sync.dma_start(bq[:], bq_d[:])
            nc.sync.dma_start(bk[:], bk_d[:])
            nc.sync.dma_start(bv[:], bv_d[:])
            nc.sync.dma_start(bg128[:], bg_d[:])
            nc.vector.memset(ones[:], 1.0)
            nc.vector.memset(c8[:], 0.125)

            # ---------------- Q/K projections (transposed out) ----------------
            # XT[c, s] = sum_d W[d, c] * xT[d, s]  (+ bias[c] via rank-1)
            QT = [acts.tile([128, S], bf16, tag=f"QT{i}", name=f"QT{i}") for i in range(2)]
            KT = [acts.tile([128, S], bf16, tag=f"KT{i}", name=f"KT{i}") for i in range(2)]
            ctxT = [acts.tile([128, S], bf16, tag=f"ctxT{i}", name=f"ctxT{i}") for i in range(2)]

            def project_T(w_tiles, x_tiles, bias, dest, cb, qh, pname):
                q0 = qh * QW
                ps = psum.tile([128, QW], f32, tag=f"sth{qh}", name=pname)
                for dc in range(ND):
                    lhsT = w_tiles[dc][:, cb * 128:(cb + 1) * 128]
                    for qc in range(QW // 512):
                        nc.tensor.matmul(
                            ps[:, qc * 512:(qc + 1) * 512],
                            lhsT,
                            x_tiles[dc][:, q0 + qc * 512:q0 + (qc + 1) * 512],
                            start=(dc == 0),
                            stop=False,
                        )
                for qc in range(QW // 512):
                    nc.tensor.matmul(
                        ps[:, qc * 512:(qc + 1) * 512],
                        bias[0:1, cb * 128:(cb + 1) * 128],
                        ones[0:1, :],
                        start=False,
                        stop=True,
                    )
                nc.vector.tensor_copy(dest[:, q0:q0 + QW], ps[:])

            for cb in range(2):
                for qh in range(NQH):
                    project_T(wq, qT, bq, QT[cb], cb, qh, f"psq{cb}{qh}")
            for cb in range(2):
                for qh in range(NQH):
                    project_T(wk, kT, bk, KT[cb], cb, qh, f"psk{cb}{qh}")

            # ---------------- gnosis gates (transposed) ----------------
            # glinT[s, h] = sum_c KT[c, s] * wg4[c, h];
            # gscT = (1 + sigmoid(glinT + bg)) / 8  -> per-partition exp scale
            gscT = [acts.tile([128, HPC], f32, tag=f"gsc{i}", name=f"gsc{i}")
                    for i in range(NSB)]
            for sb in range(NSB):
                gps = psum.tile([128, HPC], f32, tag=f"sth{sb % 2}", name=f"gps{sb}")
                for cc in range(2):
                    nc.tensor.matmul(
                        gps[:],
                        KT[cc][:, sb * 128:(sb + 1) * 128],
                        wg4c[cc][:],
                        start=(cc == 0),
                        stop=(cc == 1),
                    )
                gn = work.tile([128, HPC], f32, tag="gn", name=f"gn{sb}", bufs=2)
                nc.scalar.activation(gn[:], gps[:], Act.Sigmoid, bias=bg128[:], scale=1.0)
                # (1+gn)/8 on ACT too: keeps the exp's scale dep same-engine
                nc.scalar.activation(
                    gscT[sb][:], gn[:], Act.Identity, bias=c8[:], scale=c8[:]
                )

            # ---------------- V projection (natural layout + ones block) ----------------
            # V[s, c] tiles [128, 4*128]: head h at cols h*128..+63 = V, cols
            # h*128+64..+127 = 1.0 -> PV matmul rows 64:128 replicate the
            # softmax denominator across 64 partitions (free normalization).
            V = [acts.tile([128, HPC * 2 * DH], bf16, tag=f"V{i}", name=f"V{i}")
                 for i in range(NSB)]
            for sb in range(NSB):
                nc.vector.memset(V[sb][:], 1.0)
                ps = psum.tile([128, C], f32, tag=f"oth{sb % 2}", name=f"psv{sb}")
                for dc in range(ND):
                    nc.tensor.matmul(
                        ps[:],
                        vT[dc][:, sb * 128:(sb + 1) * 128],
                        wv[dc][:],
                        start=(dc == 0),
                        stop=False,
                    )
                nc.tensor.matmul(
                    ps[:], ones[0:1, 0:128], bv[:], start=False, stop=True
                )
                nc.vector.tensor_copy(
                    V[sb][:].rearrange("p (h x) -> p h x", h=HPC)[:, :, 0:DH],
                    ps[:].rearrange("p (h x) -> p h x", h=HPC),
                )

            # ---------------- attention: head pairs interleaved ----------------
            # Pair (h0, h1) = (2cb, 2cb+1) live on partitions 0:64 / 64:128 of
            # QT/KT tile cb -> their QK matmuls target different PE row groups
            # and run concurrently (LDWEIGHTS hidden by the reorder window).
            for cb in range(2):
                h0, h1 = 2 * cb, 2 * cb + 1
                for qh in range(NQH):
                    q0 = qh * QW
                    ota = psum.tile([128, QW], f32, tag="oth0", name=f"ota{cb}{qh}")
                    otb = psum.tile([128, QW], f32, tag="oth1", name=f"otb{cb}{qh}")
                    for kb in range(NSB):
                        sta = psum.tile([128, QW], f32, tag="sth0", name=f"sta{cb}{qh}{kb}")
                        stb = psum.tile([128, QW], f32, tag="sth1", name=f"stb{cb}{qh}{kb}")
                        kslc = slice(kb * 128, (kb + 1) * 128)
                        for po, st in ((0, sta), (64, stb)):
                            for qc in range(QW // 512):
                                cs = slice(qc * 512, (qc + 1) * 512)
                                qs = slice(q0 + qc * 512, q0 + (qc + 1) * 512)
                                nc.tensor.matmul(st[:, cs], KT[cb][po:po + 64, kslc],
                                                 QT[cb][po:po + 64, qs],
                                                 start=True, stop=True)
                        pta = work.tile([128, QW], bf16, tag="pta", name=f"pta{cb}{qh}{kb}", bufs=2)
                        ptb = work.tile([128, QW], bf16, tag="ptb", name=f"ptb{cb}{qh}{kb}", bufs=2)
                        nc.scalar.activation(pta[:], sta[:], Act.Exp,
                                             scale=gscT[kb][:, h0:h0 + 1])
                        nc.scalar.activation(ptb[:], stb[:], Act.Exp,
                                             scale=gscT[kb][:, h1:h1 + 1])
                        vh0 = V[kb][:, h0 * 2 * DH:(h0 + 1) * 2 * DH]
                        vh1 = V[kb][:, h1 * 2 * DH:(h1 + 1) * 2 * DH]
                        for qc in range(QW // 512):
                            cs = slice(qc * 512, (qc + 1) * 512)
                            nc.tensor.matmul(ota[:, cs], vh0, pta[:, cs],
                                             start=(kb == 0), stop=(kb == NSB - 1))
                        for qc in range(QW // 512):
                            cs = slice(qc * 512, (qc + 1) * 512)
                            nc.tensor.matmul(otb[:, cs], vh1, ptb[:, cs],
                                             start=(kb == 0), stop=(kb == NSB - 1))
                    # evacuate PSUM promptly (releases slots for next iter),
                    # then normalize on SBUF off the PE critical path
                    for po, ot in ((0, ota), (64, otb)):
                        otc = work.tile([128, QW], f32, tag=f"otc{po}", name=f"otc{cb}{qh}{po}", bufs=1)
                        nc.vector.tensor_copy(otc[:], ot[:])
                        rec = work.tile([DH, QW], f32, tag=f"rec{po}", name=f"rec{cb}{qh}{po}", bufs=1)
                        nc.vector.reciprocal(rec[:], otc[DH:2 * DH, :])
                        nc.vector.tensor_tensor(
                            ctxT[cb][po:po + 64, q0:q0 + QW],
                            otc[0:DH, :],
                            rec[:],
                            Alu.mult,
                        )

            # ---------------- fused output projection ----------------
            # partial[s, o] = sum_c ctxT[c, s] * Wf[c, o]
            for qb in range(NSB):
                tg = ("oth0", "oth1") if qb % 2 == 0 else ("sth0", "sth1")
                ps0 = psum.tile([128, 512], f32, tag=tg[0], name=f"pso{qb}a")
                ps1 = psum.tile([128, 512], f32, tag=tg[1], name=f"pso{qb}b")
                for cc in range(2):
                    lhsT = ctxT[cc][:, qb * 128:(qb + 1) * 128]
                    for oc, ps in enumerate((ps0, ps1)):
                        nc.tensor.matmul(
                            ps[:],
                            lhsT,
                            wf[cc][:, oc * 512:(oc + 1) * 512],
                            start=(cc == 0),
                            stop=(cc == 1),
                        )
                ob = outp.tile([128, D], f32, tag="ob", name=f"ob{qb}")
                nc.vector.tensor_copy(ob[:, 0:512], ps0[:])
                nc.vector.tensor_copy(ob[:, 512:1024], ps1[:])
                nc.sync.dma_start(out_d[qb * 128:(qb + 1) * 128, :], ob[:])

    nc.finalize()
    return nc


def get_nc():
    if "nc" not in _nc_cache:
        _nc_cache["nc"] = build_bass()
    return _nc_cache["nc"]


def make_in_maps(query, key_, value, Wq, bq, Wk, bk, Wv, bv, wg, bg, Wo, bo, Wd, bd, Wh, bh):
    """Host-side sharding: returns (in_maps for 8 cores, fused bias)."""
    f = np.asarray
    Wf = f(Wo, np.float64) @ f(Wd, np.float64) @ f(Wh, np.float64)
    bf = (f(bo, np.float64) @ f(Wd, np.float64) @ f(Wh, np.float64)
          + f(bd, np.float64) @ f(Wh, np.float64) + f(bh, np.float64))

    wg4 = np.zeros((C, HPC), np.float32)
    for h in range(HPC):
        wg4[h * DH:(h + 1) * DH, h] = np.asarray(wg, np.float32)
    wg4 = wg4.astype(BF16)
    bg128 = np.full((128, 1), np.float32(bg), np.float32)

    xT = []
    for b in range(B):
        xT.append(tuple(
            np.ascontiguousarray(np.asarray(x[b], np.float32).T).astype(BF16)
            for x in (query, key_, value)
        ))

    in_maps = []
    for c in range(NCORES):
        b, g = divmod(c, HPC)
        cols = slice(g * C, (g + 1) * C)
        qTb, kTb, vTb = xT[b]
        in_maps.append({
            "qT": qTb, "kT": kTb, "vT": vTb,
            "wq": np.ascontiguousarray(np.asarray(Wq, np.float32)[:, cols]).astype(BF16),
            "wk": np.ascontiguousarray(np.asarray(Wk, np.float32)[:, cols]).astype(BF16),
            "wv": np.ascontiguousarray(np.asarray(Wv, np.float32)[:, cols]).astype(BF16),
            "wf": np.ascontiguousarray(Wf[cols, :]).astype(BF16),
            "wg4": wg4, "bg128": bg128,
            "bq": np.asarray(bq, np.float32)[None, cols].astype(BF16),
            "bk": np.asarray(bk, np.float32)[None, cols].astype(BF16),
            "bv": np.asarray(bv, np.float32)[None, cols].astype(BF16),
        })
    return in_maps, bf.astype(np.float32)


def gather(results, bf):
    out = np.zeros((B, S, D), np.float32)
    for c in range(NCORES):
        b = c // HPC
        out[b] += results[c]["out"]
    out += bf[None, None, :]
    return out


def kernel(**inputs):
    from concourse.bass_utils import run_bass_kernel_spmd

    nc = get_nc()
    in_maps, bf = make_in_maps(**inputs)
    res = run_bass_kernel_spmd(nc, in_maps, core_ids=list(range(NCORES)))
    return gather(res.results, bf)


# revision 16
# speedup vs baseline: 1.3211x; 1.0071x over previous
"""Trainium2 Bass kernel for MultiHeadSyntonicAttention.

Problem: B=2, S=2048, D=1024, H=16 heads, DH=64.
  q/k/v = Linear(query/key/value); per-head gnosis gate
  gn = sigmoid(k . wg + bg); scores = (q k^T / sqrt(dh)) * (1+gn);
  out = softmax(scores) v;  out = ((out Wo+bo) Wd+bd) Wh+bh.

Sharding (8 cores): core c -> batch b=c//4, head-group g=c%4 (4 heads).
Each core computes its heads' attention and a row-slice partial of the
fused output projection Wf = Wo@Wd@Wh; host sums 4 partials per batch.

Device layout (everything "transposed", tokens on the free axis):
  host ships qT/kT/vT = x[b].T  [D=1024, S=2048] bf16
  QT[c,s] (c = head-local dim, 256 rows) = Wq_s^T qT     (lhsT=Wq_s nat.)
  K gating folded into K: K' = KT * (1+gn[head, s])
  ST[k,q] = K'T^T Q                      (contract dh=64)
  PT = exp(ST/8)  bf16                   (max-sub skipped; scores bounded)
  OT[0:64,q] accum over k-blocks: lhsT=[V_head | ones] -> row 64 = denom
  ctxT = OT[0:64]/OT[64]  -> partial = ctxT^T Wf_s  [2048, 1024] f32

PSUM is 8 banks x 512 f32; attention runs in q-halves of 1024 so score
tiles (2 banks) and PV accumulators (2 banks) double-buffer within 8.
"""

import sys

sys.path.insert(0, "/opt/trn_rl_repo")

import numpy as np
import ml_dtypes

BF16 = ml_dtypes.bfloat16

B, S, D, H = 2, 2048, 1024, 16
DH = D // H          # 64
HPC = 4              # heads per core
C = HPC * DH         # 256 head-local columns per core
NCORES = 8
ND = D // 128        # 8 d-chunks
NSB = S // 128       # 16 s-blocks
QW = 1024            # attention q-tile width
NQH = S // QW        # 2 q-halves

_nc_cache = {}


def build_bass():
    import concourse.bass as bass
    import concourse.mybir as mybir
    import concourse.tile as tile
    from concourse import bacc

    f32 = mybir.dt.float32
    bf16 = mybir.dt.bfloat16
    Alu = mybir.AluOpType
    Act = mybir.ActivationFunctionType

    nc = bacc.Bacc(None, target_bir_lowering=False, name="syntonic_attn")

    qT_d = nc.dram_tensor("qT", [D, S], bf16, kind="ExternalInput")
    kT_d = nc.dram_tensor("kT", [D, S], bf16, kind="ExternalInput")
    vT_d = nc.dram_tensor("vT", [D, S], bf16, kind="ExternalInput")
    wq_d = nc.dram_tensor("wq", [D, C], bf16, kind="ExternalInput")
    wk_d = nc.dram_tensor("wk", [D, C], bf16, kind="ExternalInput")
    wv_d = nc.dram_tensor("wv", [D, C], bf16, kind="ExternalInput")
    wf_d = nc.dram_tensor("wf", [C, D], bf16, kind="ExternalInput")
    wg4_d = nc.dram_tensor("wg4", [C, HPC], bf16, kind="ExternalInput")
    bq_d = nc.dram_tensor("bq", [1, C], bf16, kind="ExternalInput")
    bk_d = nc.dram_tensor("bk", [1, C], bf16, kind="ExternalInput")
    bv_d = nc.dram_tensor("bv", [1, C], bf16, kind="ExternalInput")
    bg_d = nc.dram_tensor("bg128", [128, 1], f32, kind="ExternalInput")
    out_d = nc.dram_tensor("out", [S, D], f32, kind="ExternalOutput")

    AW = 512                 # attention q-tile width (1 PSUM bank)
    NAQ = S // AW            # 4 attention q-tiles

    with tile.TileContext(nc) as tc:
        with (
            tc.tile_pool(name="res", bufs=1) as res,
            tc.tile_pool(name="acts", bufs=1) as acts,
            tc.tile_pool(name="work", bufs=3) as work,
            tc.tile_pool(name="outp", bufs=3) as outp,
            tc.tile_pool(name="psum", bufs=2, space="PSUM") as psum,
        ):
            # ---------------- resident input loads ----------------
            qT = [res.tile([128, S], bf16, tag=f"qT{i}", name=f"qT{i}") for i in range(ND)]
            kT = [res.tile([128, S], bf16, tag=f"kT{i}", name=f"kT{i}") for i in range(ND)]
            vT = [res.tile([128, S], bf16, tag=f"vT{i}", name=f"vT{i}") for i in range(ND)]
            wq = [res.tile([128, C], bf16, tag=f"wq{i}", name=f"wq{i}") for i in range(ND)]
            wk = [res.tile([128, C], bf16, tag=f"wk{i}", name=f"wk{i}") for i in range(ND)]
            wv = [res.tile([128, C], bf16, tag=f"wv{i}", name=f"wv{i}") for i in range(ND)]
            wf = [res.tile([128, D], bf16, tag=f"wf{i}", name=f"wf{i}") for i in range(2)]
            wg4 = [res.tile([128, HPC], bf16, tag=f"wg4{i}", name=f"wg4{i}") for i in range(2)]
            wg4c = [res.tile([128, HPC], bf16, tag=f"wg4c{i}", name=f"wg4c{i}")
                    for i in range(2)]
            bq = res.tile([1, C], bf16, tag="bq")
            bk = res.tile([1, C], bf16, tag="bk")
            bv = res.tile([1, C], bf16, tag="bv")
            bg128 = res.tile([128, 1], f32, tag="bg128")
            ones = res.tile([1, 512], bf16, tag="ones")
            c8 = res.tile([128, 1], f32, tag="c8")

            for i in range(ND):
                nc.sync.dma_start(wq[i][:], wq_d[i * 128:(i + 1) * 128, :])
                nc.sync.dma_start(qT[i][:], qT_d[i * 128:(i + 1) * 128, :])
            for i in range(ND):
                nc.sync.dma_start(wk[i][:], wk_d[i * 128:(i + 1) * 128, :])
                nc.sync.dma_start(kT[i][:], kT_d[i * 128:(i + 1) * 128, :])
            for i in range(ND):
                nc.sync.dma_start(wv[i][:], wv_d[i * 128:(i + 1) * 128, :])
                nc.sync.dma_start(vT[i][:], vT_d[i * 128:(i + 1) * 128, :])
            for i in range(2):
                nc.sync.dma_start(wf[i][:], wf_d[i * 128:(i + 1) * 128, :])
                nc.sync.dma_start(wg4[i][:], wg4_d[i * 128:(i + 1) * 128, :])
                nc.vector.tensor_copy(wg4c[i][:], wg4[i][:])
            nc.sync.dma_start(bq[:], bq_d[:])
            nc.sync.dma_start(bk[:], bk_d[:])
            nc.sync.dma_start(bv[:], bv_d[:])
            nc.sync.dma_start(bg128[:], bg_d[:])
            nc.vector.memset(ones[:], 1.0)
            nc.vector.memset(c8[:], 0.125)

            # ---------------- Q/K projections (transposed out) ----------------
            QT = [acts.tile([128, S], bf16, tag=f"QT{i}", name=f"QT{i}") for i in range(2)]
            KT = [acts.tile([128, S], bf16, tag=f"KT{i}", name=f"KT{i}") for i in range(2)]
            ctxT = [acts.tile([128, S], bf16, tag=f"ctxT{i}", name=f"ctxT{i}") for i in range(2)]

            pj = [0]

            def project_T(w_tiles, x_tiles, bias, dest, cb, qs, pname):
                # XT[c, q0:q0+512] = sum_d W[d, c]*xT[d, ...] + bias rank-1
                ps = psum.tile([128, AW], f32, tag=f"st{pj[0] % 2}", name=pname)
                pj[0] += 1
                for dc in range(ND):
                    nc.tensor.matmul(
                        ps[:],
                        w_tiles[dc][:, cb * 128:(cb + 1) * 128],
                        x_tiles[dc][:, qs],
                        start=(dc == 0),
                        stop=False,
                    )
                nc.tensor.matmul(
                    ps[:], bias[0:1, cb * 128:(cb + 1) * 128], ones[0:1, :],
                    start=False, stop=True,
                )
                nc.vector.tensor_copy(dest[:, qs], ps[:])

            for cb in range(2):
                for a in range(NAQ):
                    project_T(wq, qT, bq, QT[cb], cb,
                              slice(a * AW, (a + 1) * AW), f"psq{cb}{a}")
            for cb in range(2):
                for a in range(NAQ):
                    project_T(wk, kT, bk, KT[cb], cb,
                              slice(a * AW, (a + 1) * AW), f"psk{cb}{a}")

            # ---------------- gnosis gates (transposed) ----------------
            gscT = [acts.tile([128, HPC], f32, tag=f"gsc{i}", name=f"gsc{i}")
                    for i in range(NSB)]
            for sb in range(NSB):
                gps = psum.tile([128, HPC], f32, tag=f"st{sb % 2}", name=f"gps{sb}")
                for cc in range(2):
                    nc.tensor.matmul(
                        gps[:],
                        KT[cc][:, sb * 128:(sb + 1) * 128],
                        wg4c[cc][:],
                        start=(cc == 0),
                        stop=(cc == 1),
                    )
                gn = work.tile([128, HPC], f32, tag="gn", name=f"gn{sb}", bufs=2)
                nc.scalar.activation(gn[:], gps[:], Act.Sigmoid, bias=bg128[:], scale=1.0)
                nc.scalar.activation(gscT[sb][:], gn[:], Act.Identity,
                                     bias=c8[:], scale=c8[:])

            # ---------------- V projection (natural layout + ones block) ----------------
            # V tiles [128, 4*128]: head h cols h*128..+63 = V, +64..+127 = 1.0
            # -> PV rows 64:128 replicate the softmax denominator for free.
            V = [acts.tile([128, HPC * 2 * DH], bf16, tag=f"V{i}", name=f"V{i}")
                 for i in range(NSB)]
            for sb in range(NSB):
                nc.vector.memset(V[sb][:], 1.0)
                ps = psum.tile([128, C], f32, tag=f"ot{sb % 2}", name=f"psv{sb}")
                for dc in range(ND):
                    nc.tensor.matmul(
                        ps[:],
                        vT[dc][:, sb * 128:(sb + 1) * 128],
                        wv[dc][:],
                        start=(dc == 0),
                        stop=False,
                    )
                nc.tensor.matmul(ps[:], ones[0:1, 0:128], bv[:], start=False, stop=True)
                nc.vector.tensor_copy(
                    V[sb][:].rearrange("p (h x) -> p h x", h=HPC)[:, :, 0:DH],
                    ps[:].rearrange("p (h x) -> p h x", h=HPC),
                )

            # ---------------- attention: head pairs, 512-wide q tiles ----------------
            for cb in range(2):
                h0, h1 = 2 * cb, 2 * cb + 1
                for a in range(NAQ):
                    qs = slice(a * AW, (a + 1) * AW)
                    ota = psum.tile([128, AW], f32, tag="ot0", name=f"ota{cb}{a}")
                    otb = psum.tile([128, AW], f32, tag="ot1", name=f"otb{cb}{a}")
                    for kb in range(NSB):
                        kslc = slice(kb * 128, (kb + 1) * 128)
                        sta = psum.tile([128, AW], f32, tag="st0", name=f"sta{cb}{a}{kb}")
                        stb = psum.tile([128, AW], f32, tag="st1", name=f"stb{cb}{a}{kb}")
                        nc.tensor.matmul(sta[:], KT[cb][0:64, kslc],
                                         QT[cb][0:64, qs], start=True, stop=True)
                        nc.tensor.matmul(stb[:], KT[cb][64:128, kslc],
                                         QT[cb][64:128, qs], start=True, stop=True)
                        pta = work.tile([128, AW], bf16, tag="pta",
                                        name=f"pta{cb}{a}{kb}", bufs=3)
                        ptb = work.tile([128, AW], bf16, tag="ptb",
                                        name=f"ptb{cb}{a}{kb}", bufs=3)
                        nc.scalar.activation(pta[:], sta[:], Act.Exp,
                                             scale=gscT[kb][:, h0:h0 + 1])
                        nc.scalar.activation(ptb[:], stb[:], Act.Exp,
                                             scale=gscT[kb][:, h1:h1 + 1])
                        nc.tensor.matmul(
                            ota[:], V[kb][:, h0 * 2 * DH:(h0 + 1) * 2 * DH], pta[:],
                            start=(kb == 0), stop=(kb == NSB - 1))
                        nc.tensor.matmul(
                            otb[:], V[kb][:, h1 * 2 * DH:(h1 + 1) * 2 * DH], ptb[:],
                            start=(kb == 0), stop=(kb == NSB - 1))
                    # evacuate psum fast, normalize on SBUF off the PE path
                    for po, ot in ((0, ota), (64, otb)):
                        otc = work.tile([128, AW], f32, tag=f"otc{po}",
                                        name=f"otc{cb}{a}{po}", bufs=2)
                        nc.vector.tensor_copy(otc[:], ot[:])
                        rec = work.tile([DH, AW], f32, tag=f"rec{po}",
                                        name=f"rec{cb}{a}{po}", bufs=2)
                        nc.vector.reciprocal(rec[:], otc[DH:2 * DH, :])
                        nc.vector.tensor_tensor(
                            ctxT[cb][po:po + 64, qs],
                            otc[0:DH, :],
                            rec[:],
                            Alu.mult,
                        )

            # ---------------- fused output projection ----------------
            for qb in range(NSB):
                tg = ("st0", "st1") if qb % 2 == 0 else ("ot0", "ot1")
                ps0 = psum.tile([128, 512], f32, tag=tg[0], name=f"pso{qb}a")
                ps1 = psum.tile([128, 512], f32, tag=tg[1], name=f"pso{qb}b")
                for cc in range(2):
                    lhsT = ctxT[cc][:, qb * 128:(qb + 1) * 128]
                    for oc, ps in enumerate((ps0, ps1)):
                        nc.tensor.matmul(
                            ps[:],
                            lhsT,
                            wf[cc][:, oc * 512:(oc + 1) * 512],
                            start=(cc == 0),
                            stop=(cc == 1),
                        )
                ob = outp.tile([128, D], f32, tag="ob", name=f"ob{qb}")
                nc.vector.tensor_copy(ob[:, 0:512], ps0[:])
                nc.vector.tensor_copy(ob[:, 512:1024], ps1[:])
                nc.sync.dma_start(out_d[qb * 128:(qb + 1) * 128, :], ob[:])

    nc.finalize()
    return nc


def get_nc():
    if "nc" not in _nc_cache:
        _nc_cache["nc"] = build_bass()
    return _nc_cache["nc"]


def make_in_maps(query, key_, value, Wq, bq, Wk, bk, Wv, bv, wg, bg, Wo, bo, Wd, bd, Wh, bh):
    """Host-side sharding: returns (in_maps for 8 cores, fused bias)."""
    f = np.asarray
    Wf = f(Wo, np.float64) @ f(Wd, np.float64) @ f(Wh, np.float64)
    bf = (f(bo, np.float64) @ f(Wd, np.float64) @ f(Wh, np.float64)
          + f(bd, np.float64) @ f(Wh, np.float64) + f(bh, np.float64))

    wg4 = np.zeros((C, HPC), np.float32)
    for h in range(HPC):
        wg4[h * DH:(h + 1) * DH, h] = np.asarray(wg, np.float32)
    wg4 = wg4.astype(BF16)
    bg128 = np.full((128, 1), np.float32(bg), np.float32)

    xT = []
    for b in range(B):
        xT.append(tuple(
            np.ascontiguousarray(np.asarray(x[b], np.float32).T).astype(BF16)
            for x in (query, key_, value)
        ))

    in_maps = []
    for c in range(NCORES):
        b, g = divmod(c, HPC)
        cols = slice(g * C, (g + 1) * C)
        qTb, kTb, vTb = xT[b]
        in_maps.append({
            "qT": qTb, "kT": kTb, "vT": vTb,
            "wq": np.ascontiguousarray(np.asarray(Wq, np.float32)[:, cols]).astype(BF16),
            "wk": np.ascontiguousarray(np.asarray(Wk, np.float32)[:, cols]).astype(BF16),
            "wv": np.ascontiguousarray(np.asarray(Wv, np.float32)[:, cols]).astype(BF16),
            "wf": np.ascontiguousarray(Wf[cols, :]).astype(BF16),
            "wg4": wg4, "bg128": bg128,
            "bq": np.asarray(bq, np.float32)[None, cols].astype(BF16),
            "bk": np.asarray(bk, np.float32)[None, cols].astype(BF16),
            "bv": np.asarray(bv, np.float32)[None, cols].astype(BF16),
        })
    return in_maps, bf.astype(np.float32)


def gather(results, bf):
    out = np.zeros((B, S, D), np.float32)
    for c in range(NCORES):
        b = c // HPC
        out[b] += results[c]["out"]
    out += bf[None, None, :]
    return out


def kernel(**inputs):
    from concourse.bass_utils import run_bass_kernel_spmd

    nc = get_nc()
    in_maps, bf = make_in_maps(**inputs)
    res = run_bass_kernel_spmd(nc, in_maps, core_ids=list(range(NCORES)))
    return gather(res.results, bf)
